# revision 29
# baseline (speedup 1.0000x reference)
"""Trainium2 Bass kernel for nn_DGCRM_88227218194820.

The reference module's dynamic-adjacency branch (gconv_hyper / nodevec /
adp) is dead code w.r.t. the returned hidden state: due to the faithful
source bug, gconv_rnn(inp, i) == concat([inp, a*inp, a*inp], -1) @ rnn_W[i]
+ rnn_b[i] uses no adjacency, and the normalized adjacencies are deleted.
The output therefore reduces to a per-row GRU gate:

    combined = concat(x, h)                      # [.., 66]
    z  = sigmoid(combined @ Wz + bz)
    r  = sigmoid(combined @ Wr + br)
    hc = tanh(concat(x, r*h) @ Wc + bc)
    out = z*h + (1-z)*hc

with Wg folded from rnn_W: Wg = W[:66] + a*(W[66:132] + W[132:198]),
summed over the two gconv_rnn calls per gate.

Layout (per core, data-parallel over batch: 2 of 16 batches per core,
R = 2048 rows): everything lives transposed (channels on partitions) and
"group-stacked" -- rows 0:1024 (group A) on partitions 0:64, rows
1024:2048 (group B) on partitions 64:128, so every ACT/DVE op uses all
128 partitions.  Each gate matmul uses a K=128 block-diagonal bf16
weight blockdiag(Wg_h, Wg_h), which computes both groups' pre-acts in
one instruction with PSUM output already group-stacked; the 2-channel x
contribution AND the gate bias (as a constant-1 input channel)
accumulate via a K=6 block-diagonal matmul.

dtypes: matmul inputs bf16 (fp32 PE matmul is ~4x slower), PSUM
accumulation fp32, activations + gating arithmetic bf16 (fp32
tensor_tensor on the DVE has no fast mode; bf16 runs 2x), output bf16
(upcast on host).  Measured end-to-end relative error ~4e-3.

Perf structure (measured window = [walrus const-memset ~t0, program
end], which includes a fixed ~6.5us compiler postamble sweeping the
semaphore file, ~0.7us framework init, and a ~2.2us exit barrier):
 - inputs ride TWO SP-ring DMAs: aux2 (x + x-weights, 6 descriptors,
   lands ~1.5us after desc-gen) then auxh = full h^T + h-weights merged
   into ONE [128, 2816B] transfer.  HBM->SBUF here is descriptor-
   latency bound (~110-400ns/descriptor, 16 SDMA engines, ~0.7us
   completion receipt), so 134 fat descriptors beat 262 thin ones by
   ~2us.  Each extra DMA on a queue also costs a ~0.7us mid-queue
   stall (its completion descriptor drains the pipe), so no more
   splitting.
 - a PE warm-up burst of 256-col dummy matmuls bridges body start to
   aux2 arrival; the five K=6 x-projection matmuls (which need only
   aux2) bridge to auxh arrival, so the PE never idles (an idle gap
   re-gates the HAM clock to 1.2 GHz; warm transition is ~3.5-4us of
   sustained activity with run-variable phase).
 - r/z pre-activations each use one [128,1024] PSUM tile spanning two
   banks so ONE sigmoid covers both 512-col matmul blocks; the ACT
   chain (r sig, z sig, 3 tanh) is the compute-phase critical path.
 - blend restructured as out = zh + oz*hc with zh = z*h and oz = 1-z
   precomputed on the DVE while the c matmuls run, leaving only 2 DVE
   ops per block after each tanh.
 - output DMAs are FIRE-AND-FORGET: emitted after the tile context
   (post all-engine barrier) with a waiter-less semaphore, so their
   ~1.5-2us HBM-write receipts complete during the compiler postamble
   instead of gating the exit barrier.  Correct because the postamble
   outlasts the receipts by >4us and nothing reads ot before program
   end.
 - run-to-run variance is ~±1us, dominated by DMA packet scheduling
   and HAM phase.
"""

import ml_dtypes
import numpy as np

import concourse.tile as tile
from concourse import bacc, mybir
from concourse.bass_utils import run_bass_kernel_spmd

N_CORES = 8
B, N, IN_DIM, HID = 16, 1024, 2, 64
GC_ALPHA = 0.05
CIN = HID + IN_DIM          # 66
R = (B // N_CORES) * N      # 2048 rows per core
G = R // 2                  # 1024 rows per group (A/B)
BLK = 512                   # psum free-dim block
NBLK = G // BLK             # 2
# HAM clock gate: the PE runs at 1.2 GHz until it has been busy ~3.4us
# (free-running 4096-cycle activity window), then 2.4 GHz.  Cover the
# whole DMA-wait with dummy matmuls (plus the x-projection matmuls,
# which don't need h) so the h matmuls run warm.  Warm-up matmuls
# pipeline at ~213ns each (256 cols @ 1.2 GHz); 8 of them bridge the
# ~1.7us from PE start to aux2 arrival with no idle gap.
N_WARMUP_MM = 8
WARM_COLS = 256

F32 = mybir.dt.float32
BF16 = mybir.dt.bfloat16
AF = mybir.ActivationFunctionType
BF16_NP = ml_dtypes.bfloat16

_program_cache = {}


def build_program():
    # Bacc (not raw Bass): its compile() runs move_matmul_waits_to_ldweights
    # + generate_event_semaphores, which split multi-sem waits to satisfy
    # the TRN2 "at most 1 sync wait per instruction" constraint.
    nc = bacc.Bacc()
    # auxh: full h^T (bf16) + blockdiag gate weights (bf16), bitcast-
    # packed as ONE f32 transfer: 128 descriptors of 2816B instead of
    # 256 small ones (HBM->SBUF DMA is descriptor-latency bound here).
    auxh = nc.dram_tensor("auxh", [128, 704], F32, kind="ExternalInput")
    # aux2: bf16 blockdiag x+bias weights and x+ones data, bitcast-packed
    aux2 = nc.dram_tensor("aux2", [6, 704], F32, kind="ExternalInput")
    ot = nc.dram_tensor("ot", [128, G], BF16, kind="ExternalOutput")
    # Raw (non-tile) SBUF tensor so its concrete AP can feed the post-
    # context fire-and-forget output DMAs.
    OT = nc.alloc_sbuf_tensor("OT", [128, G], BF16)

    with tile.TileContext(nc) as tc:
        with (
            tc.tile_pool(name="sb", bufs=1) as sb,
            tc.tile_pool(name="ps", bufs=1, space="PSUM") as ps,
        ):
            AUXH = sb.tile([128, 704], F32, tag="AUXH")
            AUX2 = sb.tile([6, 704], F32, tag="AUX2")
            ZT = sb.tile([128, G], BF16, tag="ZT")
            RT = sb.tile([128, G], BF16, tag="RT")
            RHB = sb.tile([128, G], BF16, tag="RHB")
            HC = sb.tile([128, G], BF16, tag="HC")
            OZ = sb.tile([128, G], BF16, tag="OZ")
            ZH = sb.tile([128, G], BF16, tag="ZH")
            MC = sb.tile([128, G], BF16, tag="MC")
            WARM = sb.tile([128, WARM_COLS], BF16, tag="WARM")
            dummy = sb.tile([1, 1], F32, tag="dummy")

            HTB0 = AUXH[:, 0:256].bitcast(BF16)    # [128, 512] h^T cols 0:512
            HTB1 = AUXH[:, 256:512].bitcast(BF16)  # [128, 512] h^T cols 512:1024
            WB = AUXH[:, 512:704].bitcast(BF16)    # [128, 384]
            WX = AUX2[:, 0:192].bitcast(BF16)      # [6, 384]
            XT = AUX2[:, 192:704].bitcast(BF16)    # [6, 1024]

            nc.vector.memset(dummy, 0.0)
            # Fire the ACT table load (sigmoid_and_others, covers tanh)
            # immediately so it overlaps the input DMAs.
            nc.scalar.activation(
                out=dummy, in_=dummy, func=AF.Sigmoid, bias=dummy[0:1, 0:1]
            )

            # Both input DMAs on the SP ring, in need-order: aux2 (17KB, 6
            # descriptors) lands ~immediately after its desc-gen so the K=6
            # x-projection matmuls can run while auxh is still in flight.
            # (The ACT ring's desc-gen measured ~2x slower, so it is kept
            # off the input path.)
            nc.sync.dma_start(out=AUX2, in_=aux2[:, :])
            nc.sync.dma_start(out=AUXH, in_=auxh[:, :])

            # PE warm-up: dummy matmuls while DMAs are in flight keep the
            # HAM activity window busy so real matmuls run at 2.4 GHz.
            # They stream into a dedicated psum bank (PWARM).
            nc.vector.memset(WARM, 0.0)

            def mm_h(psum_t, g, rhs_t, cols, n=BLK):
                wc = slice(128 * g, 128 * g + 128)
                nc.tensor.matmul(
                    psum_t[:, 0:n], WB[:, wc], rhs_t[:, cols],
                    start=False, stop=True, skip_group_check=True,
                )

            def mm_xb(psum_t, g, cols, n=BLK):
                # x channels + constant-1 bias channel, K=6 blockdiag
                wc = slice(128 * g, 128 * g + 128)
                nc.tensor.matmul(
                    psum_t[:, 0:n], WX[0:6, wc], XT[0:6, cols],
                    start=True, stop=False, skip_group_check=True,
                )

            cols0 = slice(0, BLK)
            cols1 = slice(BLK, G)
            colsL = slice(0, BLK)  # local cols within second-half tile
            half = BLK // 2
            cols1a = slice(BLK, BLK + half)
            cols1b = slice(BLK + half, G)
            # r and z psums each span 2 adjacent banks (each matmul stays
            # within one) so a single [128, 1024] sigmoid covers both
            # column blocks — one ACT instruction instead of two.
            pr = ps.tile([128, G], F32, tag="pr")
            pr0 = pr[:, 0:BLK]
            pr1 = pr[:, BLK:G]
            pz = ps.tile([128, G], F32, tag="pz")
            pc0 = ps.tile([128, BLK], F32, tag="pc0")
            pc1a = ps.tile([128, half], F32, tag="pc1a")
            pc1b = ps.tile([128, half], F32, tag="pc1b")
            pwarm = ps.tile([128, WARM_COLS], F32, tag="pwarm")

            for _ in range(N_WARMUP_MM):
                nc.tensor.matmul(
                    pwarm[:, :], WARM[:, 0:128], WARM[:, :],
                    start=True, stop=True, skip_group_check=True,
                )

            # ---- flattened schedule (PE FIFO / ACT FIFO tuned) ----
            # The r x-projections (need only aux2) run right after the
            # warm-ups and dovetail into the h matmuls as auxh lands; the
            # z x-projections slot in after them to keep the PE busy with
            # no idle gap (an idle gap would re-gate the HAM clock).  Gate
            # order r, z, c gets the r sigmoids (critical path to the
            # candidate matmul) out earliest.
            mm_xb(pr0, 1, cols0)
            mm_xb(pr1, 1, cols1)
            mm_h(pr0, 1, HTB0, cols0)
            mm_h(pr1, 1, HTB1, colsL)
            nc.scalar.activation(out=RT[:, :], in_=pr[:, :], func=AF.Sigmoid)
            nc.vector.tensor_mul(RHB[:, cols0], RT[:, cols0], HTB0[:, :])
            nc.vector.tensor_mul(RHB[:, cols1], RT[:, cols1], HTB1[:, :])

            mm_xb(pz[:, cols0], 0, cols0)
            mm_xb(pz[:, cols1], 0, cols1)
            mm_h(pz[:, cols0], 0, HTB0, cols0)
            mm_h(pz[:, cols1], 0, HTB1, colsL)
            nc.scalar.activation(out=ZT[:, :], in_=pz[:, :], func=AF.Sigmoid)

            # c-gate: block 0 at 512, block 1 at 2x256 so the final tanh/
            # blend pipeline at fine granularity after the last matmul.
            mm_xb(pc0, 2, cols0)
            mm_xb(pc1a, 2, cols1a, n=half)
            mm_xb(pc1b, 2, cols1b, n=half)
            mm_h(pc0, 2, RHB, cols0)
            mm_h(pc1a, 2, RHB, cols1a, n=half)
            mm_h(pc1b, 2, RHB, cols1b, n=half)

            # Pull z's blend contribution off the tail: while the c matmuls
            # run, the DVE (idle) precomputes zh = z*h and oz = 1-z, so
            # after each tanh only 2 DVE ops remain per block:
            #   out = zh + oz*hc
            nc.vector.tensor_scalar(
                OZ[:, :], ZT[:, :], -1.0, 1.0,
                op0=mybir.AluOpType.mult, op1=mybir.AluOpType.add,
            )
            nc.vector.tensor_mul(ZH[:, cols0], ZT[:, cols0], HTB0[:, :])
            nc.vector.tensor_mul(ZH[:, cols1], ZT[:, cols1], HTB1[:, :])

            nc.scalar.activation(out=HC[:, cols0], in_=pc0[:, :], func=AF.Tanh)
            nc.scalar.activation(out=HC[:, cols1a], in_=pc1a[:, :], func=AF.Tanh)
            nc.scalar.activation(out=HC[:, cols1b], in_=pc1b[:, :], func=AF.Tanh)

            nc.vector.tensor_mul(MC[:, cols0], OZ[:, cols0], HC[:, cols0])
            nc.vector.tensor_add(OT[:, cols0], ZH[:, cols0], MC[:, cols0])
            for c in (cols1a, cols1b):
                nc.vector.tensor_mul(MC[:, c], OZ[:, c], HC[:, c])
                nc.vector.tensor_add(OT[:, c], ZH[:, c], MC[:, c])

    # Fire-and-forget output DMAs, emitted AFTER the tile context: the
    # tile-exit all-engine barrier guarantees the blends are done, and
    # nothing waits on the transfer receipts — they complete during the
    # fixed ~6us compiler postamble (semaphore-file reset) that follows,
    # instead of serializing ~1.5-2us of HBM-write receipt before it.
    # (walrus requires sync info on DGE DMAs; osem has no waiters.)
    osem = nc.alloc_semaphore("osem")
    nc.scalar.dma_start(out=ot[:, cols0], in_=OT[:, cols0]).then_inc(osem, 16)
    nc.sync.dma_start(out=ot[:, cols1], in_=OT[:, cols1]).then_inc(osem, 16)

    nc.compile()
    return nc


def get_program():
    if "nc" not in _program_cache:
        _program_cache["nc"] = build_program()
    return _program_cache["nc"]


def fold_params(rnn_W, rnn_b):
    """Fold the gconv_rnn bug + gate sums into per-gate [66,64] weights."""
    Wf = rnn_W[:, :CIN, :] + GC_ALPHA * (
        rnn_W[:, CIN : 2 * CIN, :] + rnn_W[:, 2 * CIN : 3 * CIN, :]
    )  # [6, 66, 64]
    Wg = np.stack([Wf[0] + Wf[1], Wf[2] + Wf[3], Wf[4] + Wf[5]])  # [3,66,64]
    bg = np.stack(
        [rnn_b[0] + rnn_b[1], rnn_b[2] + rnn_b[3], rnn_b[4] + rnn_b[5]]
    )  # [3, 64]
    return Wg, bg


def make_in_maps(x, h, rnn_W, rnn_b):
    Wg, bg = fold_params(rnn_W, rnn_b)
    # combined = concat(x, h): channels 0:2 are x, 2:66 are h.
    # Gate order in the packed weights: z=0, r=1, c=2.
    W_x = Wg[:, :IN_DIM, :]  # [3, 2, 64]
    W_h = Wg[:, IN_DIM:, :]  # [3, 64, 64]

    # Block-diagonal bf16 weights: gate g occupies cols 128g:128(g+1);
    # out = blockdiag(Wg_h, Wg_h).T @ [h_A; h_B] = [gate_A; gate_B].
    # wx rows per group: [x0; x1; 1] -> [Wg_x; bg] folds the bias in.
    wb_host = np.zeros((128, 384), BF16_NP)
    wx_host = np.zeros((6, 384), BF16_NP)
    for g in range(3):
        wb_host[0:64, 128 * g : 128 * g + 64] = W_h[g]
        wb_host[64:128, 128 * g + 64 : 128 * g + 128] = W_h[g]
        wx_host[0:2, 128 * g : 128 * g + 64] = W_x[g]
        wx_host[2, 128 * g : 128 * g + 64] = bg[g]
        wx_host[3:5, 128 * g + 64 : 128 * g + 128] = W_x[g]
        wx_host[5, 128 * g + 64 : 128 * g + 128] = bg[g]

    hf = h.reshape(N_CORES, R, HID)
    xf = x.reshape(N_CORES, R, IN_DIM)
    in_maps = []
    for c in range(N_CORES):
        ht_host = np.ascontiguousarray(
            np.concatenate([hf[c, :G].T, hf[c, G:].T], axis=0)
        ).astype(BF16_NP)  # [128, G] bf16
        auxh_host = np.empty((128, 704), np.float32)
        auxh_host[:, 0:512] = ht_host.view(np.float32)
        auxh_host[:, 512:704] = wb_host.view(np.float32)
        xt_host = np.empty((6, G), BF16_NP)
        xt_host[0:2] = xf[c, :G].T
        xt_host[2] = 1.0
        xt_host[3:5] = xf[c, G:].T
        xt_host[5] = 1.0
        aux2_host = np.empty((6, 704), np.float32)
        aux2_host[:, 0:192] = wx_host.view(np.float32)
        aux2_host[:, 192:704] = xt_host.view(np.float32)
        in_maps.append(dict(auxh=auxh_host, aux2=aux2_host))
    return in_maps


def gather_output(results):
    outs = []
    for c in range(N_CORES):
        o = np.asarray(results[c]["ot"]).astype(np.float32)  # [128, G]
        outs.append(np.concatenate([o[:64].T, o[64:].T], axis=0))  # [R, HID]
    return (
        np.concatenate(outs, axis=0).reshape(B, N, HID).astype(np.float32)
    )


def run(inputs, trace=False, **kw):
    x = np.ascontiguousarray(np.asarray(inputs["x"], dtype=np.float32))
    h = np.ascontiguousarray(
        np.asarray(inputs["hidden_state"], dtype=np.float32)
    )
    rnn_W = np.asarray(inputs["rnn_W"], dtype=np.float32)
    rnn_b = np.asarray(inputs["rnn_b"], dtype=np.float32)

    in_maps = make_in_maps(x, h, rnn_W, rnn_b)
    nc = get_program()
    res = run_bass_kernel_spmd(
        nc, in_maps, core_ids=list(range(N_CORES)), trace=trace, **kw
    )
    return gather_output(res.results), res


def kernel(**inputs) -> np.ndarray:
    out, _ = run(inputs)
    return out



# revision 33
# speedup vs baseline: 1.0802x; 1.0802x over previous
"""Trainium2 Bass kernel for nn_DGCRM_88227218194820.

The reference module's dynamic-adjacency branch (gconv_hyper / nodevec /
adp) is dead code w.r.t. the returned hidden state: due to the faithful
source bug, gconv_rnn(inp, i) == concat([inp, a*inp, a*inp], -1) @ rnn_W[i]
+ rnn_b[i] uses no adjacency, and the normalized adjacencies are deleted.
The output therefore reduces to a per-row GRU gate:

    combined = concat(x, h)                      # [.., 66]
    z  = sigmoid(combined @ Wz + bz)
    r  = sigmoid(combined @ Wr + br)
    hc = tanh(concat(x, r*h) @ Wc + bc)
    out = z*h + (1-z)*hc

with Wg folded from rnn_W: Wg = W[:66] + a*(W[66:132] + W[132:198]),
summed over the two gconv_rnn calls per gate.

Layout (per core, data-parallel over batch: 2 of 16 batches per core,
R = 2048 rows): everything lives transposed (channels on partitions) and
"group-stacked" -- rows 0:1024 (group A) on partitions 0:64, rows
1024:2048 (group B) on partitions 64:128, so every ACT/DVE op uses all
128 partitions.  Each gate matmul uses a K=128 block-diagonal bf16
weight blockdiag(Wg_h, Wg_h), which computes both groups' pre-acts in
one instruction with PSUM output already group-stacked; the 2-channel x
contribution AND the gate bias (as a constant-1 input channel)
accumulate via a K=6 block-diagonal matmul.

dtypes: matmul inputs bf16 (fp32 PE matmul is ~4x slower), PSUM
accumulation fp32, activations + gating arithmetic bf16 (fp32
tensor_tensor on the DVE has no fast mode; bf16 runs 2x), output bf16
(upcast on host).  Measured end-to-end relative error ~4e-3.

Perf structure (measured window = [walrus const-memset ~t0, program
end], which includes a fixed ~6.5us compiler postamble sweeping the
semaphore file, ~0.7us framework init, and a ~2.2us exit barrier):
 - inputs ride TWO SP-ring DMAs: aux2 (x + x-weights, 6 descriptors,
   lands ~1.5us after desc-gen) then auxh = full h^T + h-weights merged
   into ONE [128, 2816B] transfer.  HBM->SBUF here is descriptor-
   latency bound (~110-400ns/descriptor, 16 SDMA engines, ~0.7us
   completion receipt), so 134 fat descriptors beat 262 thin ones by
   ~2us.  Each extra DMA on a queue also costs a ~0.7us mid-queue
   stall (its completion descriptor drains the pipe), so no more
   splitting.
 - a PE warm-up burst of 256-col dummy matmuls bridges body start to
   aux2 arrival; the five K=6 x-projection matmuls (which need only
   aux2) bridge to auxh arrival, so the PE never idles (an idle gap
   re-gates the HAM clock to 1.2 GHz; warm transition is ~3.5-4us of
   sustained activity with run-variable phase).
 - r/z pre-activations each use one [128,1024] PSUM tile spanning two
   banks so ONE sigmoid covers both 512-col matmul blocks; the ACT
   chain (r sig, z sig, 3 tanh) is the compute-phase critical path.
 - blend restructured as out = zh + oz*hc with zh = z*h and oz = 1-z
   precomputed on the DVE while the c matmuls run, leaving only 2 DVE
   ops per block after each tanh.
 - output DMAs are FIRE-AND-FORGET: emitted after the tile context
   (post all-engine barrier) with a waiter-less semaphore, so their
   ~1.5-2us HBM-write receipts complete during the compiler postamble
   instead of gating the exit barrier.  Correct because the postamble
   outlasts the receipts by >4us and nothing reads ot before program
   end.
 - run-to-run variance is ~±1us, dominated by DMA packet scheduling
   and HAM phase.
"""

import ml_dtypes
import numpy as np

import concourse.tile as tile
from concourse import bacc, mybir
from concourse.bass_utils import run_bass_kernel_spmd

N_CORES = 8
B, N, IN_DIM, HID = 16, 1024, 2, 64
GC_ALPHA = 0.05
CIN = HID + IN_DIM          # 66
R = (B // N_CORES) * N      # 2048 rows per core
G = R // 2                  # 1024 rows per group (A/B)
BLK = 512                   # psum free-dim block
NBLK = G // BLK             # 2
# HAM clock gate: the PE runs at 1.2 GHz until it has been busy ~3.4us
# (free-running 4096-cycle activity window), then 2.4 GHz.  Cover the
# whole DMA-wait with dummy matmuls (plus the x-projection matmuls,
# which don't need h) so the h matmuls run warm.  Warm-up matmuls
# pipeline at ~213ns each (256 cols @ 1.2 GHz); 8 of them bridge the
# ~1.7us from PE start to aux2 arrival with no idle gap.
N_WARMUP_MM = 12
WARM_COLS = 256

F32 = mybir.dt.float32
BF16 = mybir.dt.bfloat16
AF = mybir.ActivationFunctionType
BF16_NP = ml_dtypes.bfloat16

_program_cache = {}


def build_program():
    # Bacc (not raw Bass): its compile() runs move_matmul_waits_to_ldweights
    # + generate_event_semaphores, which split multi-sem waits to satisfy
    # the TRN2 "at most 1 sync wait per instruction" constraint.
    nc = bacc.Bacc()
    # auxh: full h^T (bf16) + blockdiag gate weights (bf16), bitcast-
    # packed as ONE f32 transfer: 128 descriptors of 2816B instead of
    # 256 small ones (HBM->SBUF DMA is descriptor-latency bound here).
    auxh = nc.dram_tensor("auxh", [128, 704], F32, kind="ExternalInput")
    # aux2: bf16 blockdiag x+bias weights and x+ones data, bitcast-packed
    aux2 = nc.dram_tensor("aux2", [6, 704], F32, kind="ExternalInput")
    ot = nc.dram_tensor("ot", [128, G], BF16, kind="ExternalOutput")
    # Raw (non-tile) SBUF tensor so its concrete AP can feed the post-
    # context fire-and-forget output DMAs.
    OT = nc.alloc_sbuf_tensor("OT", [128, G], BF16)

    with tile.TileContext(nc) as tc:
        with (
            tc.tile_pool(name="sb", bufs=1) as sb,
            tc.tile_pool(name="ps", bufs=1, space="PSUM") as ps,
        ):
            AUXH = sb.tile([128, 704], F32, tag="AUXH")
            AUX2 = sb.tile([6, 704], F32, tag="AUX2")
            ZT = sb.tile([128, G], BF16, tag="ZT")
            RT = sb.tile([128, G], BF16, tag="RT")
            RHB = sb.tile([128, G], BF16, tag="RHB")
            HC = sb.tile([128, G], BF16, tag="HC")
            OZ = sb.tile([128, G], BF16, tag="OZ")
            ZH = sb.tile([128, G], BF16, tag="ZH")
            MC = sb.tile([128, G], BF16, tag="MC")
            WARM = sb.tile([128, WARM_COLS], BF16, tag="WARM")
            dummy = sb.tile([1, 1], F32, tag="dummy")

            HTB0 = AUXH[:, 0:256].bitcast(BF16)    # [128, 512] h^T cols 0:512
            HTB1 = AUXH[:, 256:512].bitcast(BF16)  # [128, 512] h^T cols 512:1024
            WB = AUXH[:, 512:704].bitcast(BF16)    # [128, 384]
            WX = AUX2[:, 0:192].bitcast(BF16)      # [6, 384]
            XT = AUX2[:, 192:704].bitcast(BF16)    # [6, 1024]

            nc.vector.memset(dummy, 0.0)
            # Fire the ACT table load (sigmoid_and_others, covers tanh)
            # immediately so it overlaps the input DMAs.
            nc.scalar.activation(
                out=dummy, in_=dummy, func=AF.Sigmoid, bias=dummy[0:1, 0:1]
            )

            # Both input DMAs on the SP ring.  auxh FIRST: its desc-gen
            # starts at body t=0 and its 128 packets flow with no earlier
            # DMA's completion stall in front, so h (the critical input)
            # arrives ~1-2us earlier.  aux2's 6 packets trail right behind;
            # the r/z x-projection matmuls move after the h matmuls to
            # match (the x path has the slack, not h).
            nc.sync.dma_start(out=AUXH, in_=auxh[:, :])
            nc.sync.dma_start(out=AUX2, in_=aux2[:, :], single_packet=True)

            # PE warm-up: dummy matmuls while DMAs are in flight keep the
            # HAM activity window busy so real matmuls run at 2.4 GHz.
            # They stream into a dedicated psum bank (PWARM).
            nc.vector.memset(WARM, 0.0)

            def mm_h(psum_t, g, rhs_t, cols, n=BLK, start=False, stop=True):
                wc = slice(128 * g, 128 * g + 128)
                nc.tensor.matmul(
                    psum_t[:, 0:n], WB[:, wc], rhs_t[:, cols],
                    start=start, stop=stop, skip_group_check=True,
                )

            def mm_xb(psum_t, g, cols, n=BLK, start=True, stop=False):
                # x channels + constant-1 bias channel, K=6 blockdiag
                wc = slice(128 * g, 128 * g + 128)
                nc.tensor.matmul(
                    psum_t[:, 0:n], WX[0:6, wc], XT[0:6, cols],
                    start=start, stop=stop, skip_group_check=True,
                )

            cols0 = slice(0, BLK)
            cols1 = slice(BLK, G)
            colsL = slice(0, BLK)  # local cols within second-half tile
            half = BLK // 2
            cols1a = slice(BLK, BLK + half)
            cols1b = slice(BLK + half, G)
            # r and z psums each span 2 adjacent banks (each matmul stays
            # within one) so a single [128, 1024] sigmoid covers both
            # column blocks — one ACT instruction instead of two.
            pr = ps.tile([128, G], F32, tag="pr")
            pr0 = pr[:, 0:BLK]
            pr1 = pr[:, BLK:G]
            pz = ps.tile([128, G], F32, tag="pz")
            pc0 = ps.tile([128, BLK], F32, tag="pc0")
            pc1a = ps.tile([128, half], F32, tag="pc1a")
            pc1b = ps.tile([128, half], F32, tag="pc1b")
            pwarm = ps.tile([128, WARM_COLS], F32, tag="pwarm")

            for _ in range(N_WARMUP_MM):
                nc.tensor.matmul(
                    pwarm[:, :], WARM[:, 0:128], WARM[:, :],
                    start=True, stop=True, skip_group_check=True,
                )

            # ---- flattened schedule (PE FIFO / ACT FIFO tuned) ----
            # For r/z the h matmuls OPEN the psum group (start=True) and
            # the x-projections close it: h arrives first now, and the
            # x-projections (aux2, trailing DMA) have the slack.  Gate
            # order r, z, c gets the r sigmoids (critical path to the
            # candidate matmul) out earliest.
            mm_h(pr0, 1, HTB0, cols0, start=True, stop=False)
            mm_h(pr1, 1, HTB1, colsL, start=True, stop=False)
            mm_xb(pr0, 1, cols0, start=False, stop=True)
            mm_xb(pr1, 1, cols1, start=False, stop=True)
            nc.scalar.activation(out=RT[:, :], in_=pr[:, :], func=AF.Sigmoid)
            nc.vector.tensor_mul(RHB[:, cols0], RT[:, cols0], HTB0[:, :])
            nc.vector.tensor_mul(RHB[:, cols1], RT[:, cols1], HTB1[:, :])

            mm_h(pz[:, cols0], 0, HTB0, cols0, start=True, stop=False)
            mm_h(pz[:, cols1], 0, HTB1, colsL, start=True, stop=False)
            mm_xb(pz[:, cols0], 0, cols0, start=False, stop=True)
            mm_xb(pz[:, cols1], 0, cols1, start=False, stop=True)
            nc.scalar.activation(out=ZT[:, :], in_=pz[:, :], func=AF.Sigmoid)

            # c-gate: block 0 at 512, block 1 at 2x256 so the final tanh/
            # blend pipeline at fine granularity after the last matmul.
            mm_xb(pc0, 2, cols0)
            mm_xb(pc1a, 2, cols1a, n=half)
            mm_xb(pc1b, 2, cols1b, n=half)
            mm_h(pc0, 2, RHB, cols0)
            mm_h(pc1a, 2, RHB, cols1a, n=half)
            mm_h(pc1b, 2, RHB, cols1b, n=half)

            # Pull z's blend contribution off the tail: while the c matmuls
            # run, the DVE (idle) precomputes zh = z*h and oz = 1-z, so
            # after each tanh only 2 DVE ops remain per block:
            #   out = zh + oz*hc
            nc.vector.tensor_scalar(
                OZ[:, :], ZT[:, :], -1.0, 1.0,
                op0=mybir.AluOpType.mult, op1=mybir.AluOpType.add,
            )
            nc.vector.tensor_mul(ZH[:, cols0], ZT[:, cols0], HTB0[:, :])
            nc.vector.tensor_mul(ZH[:, cols1], ZT[:, cols1], HTB1[:, :])

            nc.scalar.activation(out=HC[:, cols0], in_=pc0[:, :], func=AF.Tanh)
            nc.scalar.activation(out=HC[:, cols1a], in_=pc1a[:, :], func=AF.Tanh)
            nc.scalar.activation(out=HC[:, cols1b], in_=pc1b[:, :], func=AF.Tanh)

            nc.vector.tensor_mul(MC[:, cols0], OZ[:, cols0], HC[:, cols0])
            nc.vector.tensor_add(OT[:, cols0], ZH[:, cols0], MC[:, cols0])
            for c in (cols1a, cols1b):
                nc.vector.tensor_mul(MC[:, c], OZ[:, c], HC[:, c])
                nc.vector.tensor_add(OT[:, c], ZH[:, c], MC[:, c])

    # Fire-and-forget output DMAs, emitted AFTER the tile context: the
    # tile-exit all-engine barrier guarantees the blends are done, and
    # nothing waits on the transfer receipts — they complete during the
    # fixed ~6us compiler postamble (semaphore-file reset) that follows,
    # instead of serializing ~1.5-2us of HBM-write receipt before it.
    # (walrus requires sync info on DGE DMAs; osem has no waiters.)
    osem = nc.alloc_semaphore("osem")
    nc.scalar.dma_start(out=ot[:, cols0], in_=OT[:, cols0]).then_inc(osem, 16)
    nc.sync.dma_start(out=ot[:, cols1], in_=OT[:, cols1]).then_inc(osem, 16)

    nc.compile()
    return nc


def get_program():
    if "nc" not in _program_cache:
        _program_cache["nc"] = build_program()
    return _program_cache["nc"]


def fold_params(rnn_W, rnn_b):
    """Fold the gconv_rnn bug + gate sums into per-gate [66,64] weights."""
    Wf = rnn_W[:, :CIN, :] + GC_ALPHA * (
        rnn_W[:, CIN : 2 * CIN, :] + rnn_W[:, 2 * CIN : 3 * CIN, :]
    )  # [6, 66, 64]
    Wg = np.stack([Wf[0] + Wf[1], Wf[2] + Wf[3], Wf[4] + Wf[5]])  # [3,66,64]
    bg = np.stack(
        [rnn_b[0] + rnn_b[1], rnn_b[2] + rnn_b[3], rnn_b[4] + rnn_b[5]]
    )  # [3, 64]
    return Wg, bg


def make_in_maps(x, h, rnn_W, rnn_b):
    Wg, bg = fold_params(rnn_W, rnn_b)
    # combined = concat(x, h): channels 0:2 are x, 2:66 are h.
    # Gate order in the packed weights: z=0, r=1, c=2.
    W_x = Wg[:, :IN_DIM, :]  # [3, 2, 64]
    W_h = Wg[:, IN_DIM:, :]  # [3, 64, 64]

    # Block-diagonal bf16 weights: gate g occupies cols 128g:128(g+1);
    # out = blockdiag(Wg_h, Wg_h).T @ [h_A; h_B] = [gate_A; gate_B].
    # wx rows per group: [x0; x1; 1] -> [Wg_x; bg] folds the bias in.
    wb_host = np.zeros((128, 384), BF16_NP)
    wx_host = np.zeros((6, 384), BF16_NP)
    for g in range(3):
        wb_host[0:64, 128 * g : 128 * g + 64] = W_h[g]
        wb_host[64:128, 128 * g + 64 : 128 * g + 128] = W_h[g]
        wx_host[0:2, 128 * g : 128 * g + 64] = W_x[g]
        wx_host[2, 128 * g : 128 * g + 64] = bg[g]
        wx_host[3:5, 128 * g + 64 : 128 * g + 128] = W_x[g]
        wx_host[5, 128 * g + 64 : 128 * g + 128] = bg[g]

    hf = h.reshape(N_CORES, R, HID)
    xf = x.reshape(N_CORES, R, IN_DIM)
    in_maps = []
    for c in range(N_CORES):
        ht_host = np.ascontiguousarray(
            np.concatenate([hf[c, :G].T, hf[c, G:].T], axis=0)
        ).astype(BF16_NP)  # [128, G] bf16
        auxh_host = np.empty((128, 704), np.float32)
        auxh_host[:, 0:512] = ht_host.view(np.float32)
        auxh_host[:, 512:704] = wb_host.view(np.float32)
        xt_host = np.empty((6, G), BF16_NP)
        xt_host[0:2] = xf[c, :G].T
        xt_host[2] = 1.0
        xt_host[3:5] = xf[c, G:].T
        xt_host[5] = 1.0
        aux2_host = np.empty((6, 704), np.float32)
        aux2_host[:, 0:192] = wx_host.view(np.float32)
        aux2_host[:, 192:704] = xt_host.view(np.float32)
        in_maps.append(dict(auxh=auxh_host, aux2=aux2_host))
    return in_maps


def gather_output(results):
    outs = []
    for c in range(N_CORES):
        o = np.asarray(results[c]["ot"]).astype(np.float32)  # [128, G]
        outs.append(np.concatenate([o[:64].T, o[64:].T], axis=0))  # [R, HID]
    return (
        np.concatenate(outs, axis=0).reshape(B, N, HID).astype(np.float32)
    )


def run(inputs, trace=False, **kw):
    x = np.ascontiguousarray(np.asarray(inputs["x"], dtype=np.float32))
    h = np.ascontiguousarray(
        np.asarray(inputs["hidden_state"], dtype=np.float32)
    )
    rnn_W = np.asarray(inputs["rnn_W"], dtype=np.float32)
    rnn_b = np.asarray(inputs["rnn_b"], dtype=np.float32)

    in_maps = make_in_maps(x, h, rnn_W, rnn_b)
    nc = get_program()
    res = run_bass_kernel_spmd(
        nc, in_maps, core_ids=list(range(N_CORES)), trace=trace, **kw
    )
    return gather_output(res.results), res


def kernel(**inputs) -> np.ndarray:
    out, _ = run(inputs)
    return out

